# revision 39
# baseline (speedup 1.0000x reference)
"""Trainium2 Bass kernel for causal multi-head attention with RoPE.

Problem: B=2, S=2048, D=2048, H=16 heads (HD=128), fp32 reference.

Sharding (8 NeuronCores): 2-way batch x 4-way heads. Core c handles
batch c//4 and heads 4*(c%4) .. 4*(c%4)+4. Each core computes a partial
output projection over its 512-wide head slice; the host sums the 4
partials per batch element (the row-parallel wo all-reduce).

Measured dead ends (kept for future reference): fp8/DoubleRow on any
value path exceeds the 2e-2 gate (V-only fp8 sims at 0.032 rel err -
softmax concentration defeats error averaging); gpsimd tensor ops and
gpsimd partition_all_reduce are ~3.5us per [128,512] op (too slow for
the den or rope muls); PE warmup matmuls are a wash (the startup is
DMA-ramp-bound, cold-clock matmuls there cost nothing); V-first filler
order and finer startup pieces both measured slower. Run-to-run note:
back-to-back benches trigger a P0 downclock (+60us!) - measure after
a ~60s idle.

Round 3 (cross-phase interleave, vs the ~333us round 2):
  The attention phases are ACT-bound: each 512-wide score block costs
  the PE 432ns (QK^T + PV) but the exp costs ACT ~600ns, so the PE
  leaked ~170ns per block (~96 off-diagonal blocks) plus a 2-3us
  bubble at every chunk boundary waiting for the last head's softmax
  tail. Fix: the NEXT chunk's Q/K/V projection chains (pure PE work)
  are emitted as ~2-matmul "doses" BETWEEN attention blocks, making
  every attention phase PE-bound. Chunk 3's attention has no next
  chunk, so chunk 2's deferred output projection fills it instead.
  Emission order: proj(0) | attn(0)*proj(1) | wo(0) | attn(1)*proj(2)
  | wo(1) | attn(2)*proj(3) | attn(3)*wo(2) | wo(3). x images are
  prefetched two chunks ahead (xpool bufs=2 holds current+next).

Round 2 changes (vs the ~335-339us revision):
  - Denominator merge: the off-diagonal DVE pre-sums fold into the
    diagonal p4d accumulator, so ONE ones-matmul per head-chunk makes
    the whole softmax denominator (-12 PE matmuls, ~2.6us).
  - Startup: rope constants ride the Scalar DMA ring (bypassing the
    Sync FIFO), the first wq/x pieces are 256KB so the first matmul
    issues earlier, and chunk 0 emits all Q chains before any K chain
    so wk/wv/wo stream in behind the Q work.
  - Rope swaps issue from the GpSimd software-DGE ring: the Sync queue
    only carries bulk loads + output rows.
  - Output rows written as two half-rows the moment their evacuations
    land; the last chunk's halves round-robin over the three DMA rings
    (Sync/Scalar/GpSimd) so the final ~2MB drains in parallel.

Baseline notes that still apply: all inputs repacked host-side into
SBUF-image layouts (4-16KB contiguous DMA lines); wq/wk head-major;
causal trim of the diagonal super-blocks; softmax denominator via a
ones-matmul (broadcasts the colsum across partitions); no PE warmup
matmuls (counterproductive - the DMA-bound startup re-throttles HAM).
"""

import math

import numpy as np
import ml_dtypes

import concourse.bass as bass
import concourse.mybir as mybir
import concourse.tile as tile
from concourse import bacc, bass_isa, bass_utils

BF16 = ml_dtypes.bfloat16
F32 = mybir.dt.float32
BF = mybir.dt.bfloat16

B, S, D, H = 2, 2048, 2048, 16
HD = 128
NCORE = 8
HPC = 4            # heads per core
OSL = HPC * HD     # 512-wide output slice per core
NT = S // 128      # 16 token tiles
ND = D // 128      # 16 contraction tiles
NCH = 4            # 512-wide token chunks
SCALE = 1.0 / math.sqrt(HD)


def _build_program():
    nc = bacc.Bacc(
        "TRN2",
        target_bir_lowering=False,
        debug=False,
        enable_asserts=False,
        num_devices=NCORE,
    )
    xr = nc.dram_tensor("xr", [4 * 128, ND * 512], BF, kind="ExternalInput").ap()
    wqr = nc.dram_tensor("wqr", [128, ND * 512], BF, kind="ExternalInput").ap()
    wkr = nc.dram_tensor("wkr", [128, ND * 512], BF, kind="ExternalInput").ap()
    wvr = nc.dram_tensor("wvr", [128, ND * 512], BF, kind="ExternalInput").ap()
    wor = nc.dram_tensor("wor", [128, HPC * D], BF, kind="ExternalInput").ap()
    cos2 = nc.dram_tensor("cos2", [128, S], BF, kind="ExternalInput").ap()
    sin2 = nc.dram_tensor("sin2", [128, S], BF, kind="ExternalInput").ap()
    trim = nc.dram_tensor("trim", [128, 128], BF, kind="ExternalInput").ap()
    out = nc.dram_tensor("out", [S, D], BF, kind="ExternalOutput").ap()

    with tile.TileContext(nc) as tc:
        _kernel_body(tc, xr, wqr, wkr, wvr, wor, cos2, sin2, trim, out)
    nc.compile()
    return nc


def _kernel_body(tc, xr, wqr, wkr, wvr, wor, cos2, sin2, trim, out):
    nc = tc.nc
    Exp = mybir.ActivationFunctionType.Exp

    with (
        tc.tile_pool(name="weights", bufs=1) as wpool,
        tc.tile_pool(name="kv", bufs=1) as kvpool,
        tc.tile_pool(name="consts", bufs=1) as cpool,
        tc.tile_pool(name="qchunk", bufs=2) as qpool,
        tc.tile_pool(name="ctxsb", bufs=2) as ctxpool,
        tc.tile_pool(name="xtp", bufs=2) as xpool,
        tc.tile_pool(name="rope", bufs=3) as rpool,
        tc.tile_pool(name="pg", bufs=6) as ppool,
        tc.tile_pool(name="pgs", bufs=2) as pspool,
        tc.tile_pool(name="den", bufs=2) as dpool,
        tc.tile_pool(name="smallsb", bufs=3) as spool,
        tc.tile_pool(name="outsb", bufs=2) as outpool,
        tc.tile_pool(name="peps", bufs=3, space="PSUM") as peps,
        tc.tile_pool(name="scps", bufs=2, space="PSUM") as scpool,
        tc.tile_pool(name="denps", bufs=1, space="PSUM") as denpool,
        tc.tile_pool(name="ctxps", bufs=2, space="PSUM") as ctxps_pool,
    ):
        wq_s = wpool.tile([128, ND * 512], BF, tag="wq")
        wk_s = wpool.tile([128, ND * 512], BF, tag="wk")
        wv_s = wpool.tile([128, ND * 512], BF, tag="wv")
        wo_s = wpool.tile([128, HPC * D], BF, tag="wo")
        cos_s = cpool.tile([128, S], BF, tag="cos")
        sin_s = cpool.tile([128, S], BF, tag="sin")
        tri_s = cpool.tile([128, 128], BF, tag="trim")
        # ones matrix: den matmul broadcasts the k-colsum to all 128
        # partitions, so no cross-partition broadcast hop is needed.
        ones_s = cpool.tile([128, 128], BF, tag="ones")
        nc.gpsimd.memset(ones_s[:], 1.0)
        # NOTE: no PE warmup matmuls - measured a wash: the startup is
        # DMA-ramp-bound, so the cold (1.2GHz) PE costs nothing there.
        kt = [kvpool.tile([128, S], BF, tag=f"kt{h}", name=f"kt{h}")
              for h in range(HPC)]
        v_s = kvpool.tile([128, NT * 512], BF, tag="v")

        xts = {}
        qts = {qc: [] for qc in range(NCH)}
        ctxTs = {qc: [] for qc in range(NCH)}
        atts = {qc: {} for qc in range(NCH)}

        def xt_dma(qc):
            xt = xpool.tile([128, ND * 512], BF, tag="xt", name=f"xt{qc}")
            xts[qc] = xt
            nc.sync.dma_start(xt[:], xr[qc * 128:(qc + 1) * 128, :])

        def proj_doses(qc, v_first=False):
            """Q/K/V projections + rope for chunk qc, yielded in ~2-MM
            doses so they can interleave the previous chunk's attention
            blocks (pure-PE filler for the ACT-bound softmax). As
            filler (v_first), the V chains go first: their doses carry
            no rope work, so the DVE/ACT-saturated attention window
            gets pure-PE filler and the rope-heavy Q/K tails land in
            the post-attention drain instead."""
            xt = xts[qc]
            ch = slice(qc * 512, (qc + 1) * 512)
            if v_first:
                yield from _v_doses(qc, xt)
            for is_q in (True, False):
                for m in range(HPC):
                    w_s = wq_s if is_q else wk_s
                    nm = "q" if is_q else "k"
                    ps = peps.tile([128, 512], F32, tag="peps",
                                   name=f"ps{nm}{qc}_{m}")
                    for d2 in range(ND // 2):
                        for d in (2 * d2, 2 * d2 + 1):
                            nc.tensor.matmul(
                                ps[:],
                                w_s[:, m * 2048 + d * 128:
                                    m * 2048 + (d + 1) * 128],
                                xt[:, d * 512:(d + 1) * 512],
                                start=(d == 0), stop=(d == ND - 1))
                        yield
                    raw = rpool.tile([128, 512], BF, tag="rraw",
                                     name=f"raw{nm}{qc}_{m}")
                    nc.scalar.copy(raw[:], ps[:])
                    # rope rotate-half: swap 64-partition halves via the
                    # (otherwise idle) GpSimd software-DGE ring - swaps
                    # on the Sync ring head-block the bulk-load FIFO
                    # (measured +20us). The swaps all finish mid-kernel,
                    # so the slow gpsimd teardown DRAIN (~6.5us sweep)
                    # overlaps the tail write drain as long as no LATE
                    # DMA rides gpsimd (hence 2-ring tail writes).
                    swp = rpool.tile([128, 512], BF, tag="rswp",
                                     name=f"swp{nm}{qc}_{m}")
                    nc.gpsimd.dma_start(swp[0:64, :], raw[64:128, :])
                    nc.gpsimd.dma_start(swp[64:128, :], raw[0:64, :])
                    t1 = rpool.tile([128, 512], BF, tag="rt1")
                    nc.vector.tensor_mul(t1[:], raw[:], cos_s[:, ch])
                    nc.vector.tensor_mul(swp[:], swp[:], sin_s[:, ch])
                    if is_q:
                        dst = qpool.tile([128, 512], BF, tag=f"qt{m}",
                                         name=f"qt{m}_{qc}")
                        qts[qc].append(dst)
                        nc.vector.tensor_add(dst[:], t1[:], swp[:])
                    else:
                        nc.vector.tensor_add(kt[m][:, ch], t1[:], swp[:])
                    yield
            if not v_first:
                yield from _v_doses(qc, xt)

        def _v_doses(qc, xt):
            for tt in range(4):
                pv = peps.tile([128, 512], F32, tag="peps",
                               name=f"psv{qc}_{tt}")
                for d2 in range(ND // 2):
                    for d in (2 * d2, 2 * d2 + 1):
                        nc.tensor.matmul(
                            pv[:],
                            xt[:, d * 512 + tt * 128:d * 512 + (tt + 1) * 128],
                            wv_s[:, d * 512:(d + 1) * 512],
                            start=(d == 0), stop=(d == ND - 1))
                    yield
                j = qc * 4 + tt
                nc.vector.tensor_copy(v_s[:, j * 512:(j + 1) * 512], pv[:])
                yield

        def emit_off(qc, h, pump):
            # off-diagonal k-tiles (full 512-wide, no mask)
            noff = 4 * qc
            ctx_ps = ctxps_pool.tile([128, 512], F32, tag="ctxps",
                                     name=f"ctxps{h}_{qc}")
            qt = qts[qc]
            pg_hist = {}
            pair = [None, None]
            accq = None
            for j in range(noff):
                sc = scpool.tile([128, 512], F32, tag="sc",
                                 name=f"sc{h}_{qc}_{j}")
                nc.tensor.matmul(
                    sc[:], kt[h][:, j * 128:(j + 1) * 128],
                    qt[h][:], start=True, stop=True)
                pg = ppool.tile([128, 512], BF, tag="pg",
                                name=f"pg{h}_{qc}_{j}")
                nc.scalar.activation(pg[:], sc[:], Exp, scale=SCALE)
                # the filler dose sits BETWEEN the scores matmul and the
                # PV matmul in the PE queue, so the exp latency is hidden
                # behind it instead of stalling the in-order PE
                pump()
                nc.tensor.matmul(
                    ctx_ps[:],
                    v_s[:, j * 512 + h * 128:j * 512 + (h + 1) * 128],
                    pg[:], start=(j == 0), stop=False)
                # denominator pre-sums on DVE: pairs -> quads -> fold
                pg_hist[j] = pg
                if j % 2 == 1:
                    pp = pspool.tile([128, 512], BF,
                                     tag=f"pgs{(j // 2) % 2}",
                                     name=f"pgs{h}_{qc}_{j}")
                    nc.vector.tensor_add(pp[:], pg_hist[j - 1][:], pg[:])
                    pair[(j // 2) % 2] = pp
                if j % 4 == 3:
                    p4 = pspool.tile([128, 512], BF, tag="pgs4",
                                     name=f"pgs4_{h}_{qc}_{j}")
                    nc.vector.tensor_add(p4[:], pair[0][:], pair[1][:])
                    if accq is None:
                        accq = p4
                    else:
                        nacc = pspool.tile([128, 512], BF, tag="pgs8",
                                           name=f"pgs8_{h}_{qc}_{j}")
                        nc.vector.tensor_add(nacc[:], accq[:], p4[:])
                        accq = nacc
            atts[qc][h] = (ctx_ps, accq)

        def emit_diag(qc, h, pump):
            # diagonal k-tiles, causally trimmed: k-tile r only sees
            # q columns >= 128r; the first 128 of those are the
            # triangular boundary block (element mask).
            noff = 4 * qc
            ctx_ps, accq = atts[qc][h]
            qt = qts[qc]
            den_ps = denpool.tile([128, 512], F32, tag="den",
                                  name=f"den{h}_{qc}")
            p4d = dpool.tile([128, 512], BF, tag="p4d",
                             name=f"p4d{h}_{qc}")
            for r in range(4):
                off = 128 * r
                j = noff + r
                sc = scpool.tile([128, 512], F32, tag="sc",
                                 name=f"scd{h}_{qc}_{r}")
                nc.tensor.matmul(
                    sc[:, off:512], kt[h][:, j * 128:(j + 1) * 128],
                    qt[h][:, off:512], start=True, stop=True)
                pg = ppool.tile([128, 512], BF, tag="pg",
                                name=f"pgd{h}_{qc}_{r}")
                nc.scalar.activation(pg[:, off:512], sc[:, off:512],
                                     Exp, scale=SCALE)
                nc.vector.tensor_mul(pg[:, off:off + 128],
                                     pg[:, off:off + 128], tri_s[:])
                pump()
                nc.tensor.matmul(
                    ctx_ps[:, off:512],
                    v_s[:, j * 512 + h * 128:j * 512 + (h + 1) * 128],
                    pg[:, off:512],
                    start=(qc == 0 and r == 0), stop=(r == 3))
                if r == 0:
                    # fold the off-diagonal DVE pre-sums in here so a
                    # single ones-matmul produces the whole denominator
                    if accq is None:
                        nc.vector.tensor_copy(p4d[:], pg[:])
                    else:
                        nc.vector.tensor_add(p4d[:], accq[:], pg[:])
                else:
                    nc.vector.tensor_add(p4d[:, off:512],
                                         p4d[:, off:512], pg[:, off:512])
            nc.tensor.matmul(den_ps[:], ones_s[:], p4d[:],
                             start=True, stop=True)
            # softmax normalization folded into ctx eviction; den is
            # already broadcast across partitions by the ones matmul
            rbc = spool.tile([128, 512], F32, tag="rbc")
            nc.vector.reciprocal_approx_fast(rbc[:], den_ps[:])
            ctx = ctxpool.tile([128, 512], BF, tag=f"ctx{h}",
                               name=f"ctxT{h}_{qc}")
            ctxTs[qc].append(ctx)
            nc.vector.tensor_mul(ctx[:], ctx_ps[:], rbc[:])
            pump()
            pump()

        # only hardware-DGE rings: a single gpsimd (software-DGE) DMA
        # costs ~6.5us of teardown DRAIN at kernel end
        rings = [nc.sync, nc.scalar]

        def emit_wo(qc):
            # staged output projection: each group's first 3 head-MMs
            # are emitted STAG groups ahead of its final head-3 MM. On
            # the last chunk the row halves round-robin over the three
            # DMA rings so the final ~2MB drains in parallel.
            ctxT = ctxTs[qc]
            STAG = 4 if qc < NCH - 1 else 5
            groups = [(tl, dc) for tl in range(4) for dc in range(4)]
            opst = {}
            osbt = {}
            nring = [0]

            def write_half(tt, half, osb):
                cs = slice(half * 1024, (half + 1) * 1024)
                if qc == NCH - 1 and tt == 4 * qc + 3:
                    # final row: partition-split each half across both
                    # rings so the very last piece drains in ~half the
                    # time (drain cost is per-packet, 128 lines/half)
                    nc.sync.dma_start(
                        out[tt * 128:tt * 128 + 64, cs], osb[0:64, cs])
                    nc.scalar.dma_start(
                        out[tt * 128 + 64:(tt + 1) * 128, cs],
                        osb[64:128, cs])
                    return
                eng = rings[nring[0] % 2] if qc == NCH - 1 else nc.sync
                nring[0] += 1
                eng.dma_start(
                    out[tt * 128:(tt + 1) * 128, cs], osb[:, cs])

            for gi in range(16 + STAG):
                if gi >= STAG:
                    tl, dc = groups[gi - STAG]
                    tt = 4 * qc + tl
                    ops = opst[gi - STAG]
                    osb = osbt[tl]
                    nc.tensor.matmul(
                        ops[:], ctxT[3][:, tl * 128:(tl + 1) * 128],
                        wo_s[:, 3 * D + dc * 512:3 * D + (dc + 1) * 512],
                        start=False, stop=True)
                    if dc % 2 == 0:
                        nc.vector.tensor_copy(
                            osb[:, dc * 512:(dc + 1) * 512], ops[:])
                    else:
                        nc.scalar.copy(
                            osb[:, dc * 512:(dc + 1) * 512], ops[:])
                    if dc == 1:
                        write_half(tt, 0, osb)
                    elif dc == 3:
                        write_half(tt, 1, osb)
                if gi < 16:
                    tl, dc = groups[gi]
                    if dc == 0:
                        osbt[tl] = outpool.tile([128, D], BF, tag="osb",
                                                name=f"osb{4 * qc + tl}")
                    # pool choice by release time: peps is free at wo
                    # start; sc and ctx tiles release progressively
                    # through the diag-3 tail.
                    if qc == NCH - 1:
                        sel = (peps, "peps") if gi % 3 == 0 else (
                            (scpool, "sc") if gi % 3 == 1 else
                            (ctxps_pool, "ctxps"))
                    elif gi in (0, 12, 14):
                        # first and last-even groups ride the proj pool:
                        # sc/ctx tiles held by the LAST wo groups would
                        # otherwise stall the next chunk's first
                        # attention allocations by ~1us. (gi15 must NOT
                        # take a 3rd late peps buf - its alloc would
                        # wait gi12's e3 matmul, which is emitted later:
                        # cross-engine deadlock.)
                        sel = (peps, "peps")
                    elif gi == 13:
                        sel = (scpool, "sc")
                    elif gi == 15:
                        sel = (ctxps_pool, "ctxps")
                    elif gi % 2 == 0:
                        sel = (scpool, "sc")
                    else:
                        sel = (ctxps_pool, "ctxps")
                    ops = sel[0].tile([128, 512], F32, tag=sel[1],
                                      name=f"ops{qc}_{tl}_{dc}")
                    opst[gi] = ops
                    for e in range(3):
                        nc.tensor.matmul(
                            ops[:], ctxT[e][:, tl * 128:(tl + 1) * 128],
                            wo_s[:, e * D + dc * 512:e * D + (dc + 1) * 512],
                            start=(e == 0), stop=False)

        def wo_doses(qc):
            """Output projection of chunk qc in ~2-MM doses (filler for
            the LAST chunk's attention, whose next-proj doesn't exist).
            All evacuations ride the DVE - ACT is saturated with exps.
            ctx for all 4 heads is long since ready, so each group runs
            its 4 MMs straight through on the (now idle) proj pool."""
            ctxT = ctxTs[qc]
            for tl in range(4):
                osb = outpool.tile([128, D], BF, tag="osb",
                                   name=f"osb{4 * qc + tl}")
                tt = 4 * qc + tl
                for dc in range(4):
                    ops = peps.tile([128, 512], F32, tag="peps",
                                    name=f"ops{qc}_{tl}_{dc}")
                    for e in range(4):
                        nc.tensor.matmul(
                            ops[:], ctxT[e][:, tl * 128:(tl + 1) * 128],
                            wo_s[:, e * D + dc * 512:e * D + (dc + 1) * 512],
                            start=(e == 0), stop=(e == 3))
                        yield
                    nc.vector.tensor_copy(
                        osb[:, dc * 512:(dc + 1) * 512], ops[:])
                    if dc == 1:
                        nc.sync.dma_start(
                            out[tt * 128:(tt + 1) * 128, 0:1024],
                            osb[:, 0:1024])
                    elif dc == 3:
                        nc.sync.dma_start(
                            out[tt * 128:(tt + 1) * 128, 1024:2048],
                            osb[:, 1024:2048])
                    yield

        # ---- startup loads -----------------------------------------
        # Small leading pieces get the first matmul started early; the
        # rope constants ride the Scalar DMA ring, bypassing the Sync
        # FIFO of bulk loads. Q chains are emitted before K chains so
        # wk/wv/wo/x1 can stream in behind the Q work.
        xt0 = xpool.tile([128, ND * 512], BF, tag="xt", name="xt0")
        xts[0] = xt0
        nc.scalar.dma_start(cos_s[:], cos2[:])
        nc.scalar.dma_start(sin_s[:], sin2[:])
        nc.scalar.dma_start(tri_s[:], trim[:])
        nc.sync.dma_start(wq_s[:, 0:1024], wqr[:, 0:1024])
        nc.sync.dma_start(xt0[:, 0:1024], xr[0:128, 0:1024])
        nc.sync.dma_start(wq_s[:, 1024:2048], wqr[:, 1024:2048])
        nc.sync.dma_start(xt0[:, 1024:2048], xr[0:128, 1024:2048])
        nc.sync.dma_start(xt0[:, 2048:4096], xr[0:128, 2048:4096])
        nc.sync.dma_start(wq_s[:, 2048:4096], wqr[:, 2048:4096])
        nc.sync.dma_start(xt0[:, 4096:6144], xr[0:128, 4096:6144])
        nc.sync.dma_start(xt0[:, 6144:8192], xr[0:128, 6144:8192])
        nc.sync.dma_start(wq_s[:, 4096:6144], wqr[:, 4096:6144])
        nc.sync.dma_start(wq_s[:, 6144:8192], wqr[:, 6144:8192])
        nc.sync.dma_start(wk_s[:, 0:2048], wkr[:, 0:2048])
        nc.sync.dma_start(wk_s[:, 2048:4096], wkr[:, 2048:4096])
        nc.sync.dma_start(wv_s[:, 0:4096], wvr[:, 0:4096])
        nc.sync.dma_start(wk_s[:, 4096:6144], wkr[:, 4096:6144])
        nc.sync.dma_start(wk_s[:, 6144:8192], wkr[:, 6144:8192])
        nc.sync.dma_start(wv_s[:, 4096:8192], wvr[:, 4096:8192])
        nc.sync.dma_start(wo_s[:], wor[:])
        xt_dma(1)

        # ---- chunk 0 projections: bulk, DMA-paced ------------------
        for _ in proj_doses(0):
            pass

        # ---- pipelined chunks --------------------------------------
        for qc in range(NCH):
            if qc < NCH - 1:
                if qc + 2 < NCH:
                    xt_dma(qc + 2)
                filler = proj_doses(qc + 1)
            else:
                filler = wo_doses(2)

            def pump(f=filler):
                next(f, None)

            for h in range(HPC):
                emit_off(qc, h, pump)
                emit_diag(qc, h, pump)
            for _ in filler:
                pass
            if qc != 2:
                emit_wo(qc)


def _host_prep(x, freqs_cos, freqs_sin, mask, wq, wk, wv, wo):
    """Build per-core input dicts (SBUF-image layouts, bf16)."""
    x = np.asarray(x, np.float32)
    wq = np.asarray(wq, np.float32)
    wk = np.asarray(wk, np.float32)
    wv = np.asarray(wv, np.float32)
    wo = np.asarray(wo, np.float32)
    cos = np.asarray(freqs_cos, np.float32)
    sin = np.asarray(freqs_sin, np.float32)

    perm = np.concatenate([np.arange(0, HD, 2), np.arange(1, HD, 2)])
    cos2 = np.ascontiguousarray(
        np.concatenate([cos.T, cos.T], axis=0)).astype(BF16)
    sin2 = np.ascontiguousarray(
        np.concatenate([-sin.T, sin.T], axis=0)).astype(BF16)

    # triangular boundary mask: T[kl, ql] = 1 iff ql >= kl
    kl = np.arange(128)[:, None]
    ql = np.arange(128)[None, :]
    trim = (ql >= kl).astype(np.float32).astype(BF16)

    def img_dxk(wT):
        # [D, K] -> [128, ND*K] with [p, d*K+c] = wT[d*128+p, c]
        Dd, K = wT.shape
        return np.ascontiguousarray(
            wT.reshape(ND, 128, K).transpose(1, 0, 2).reshape(128, ND * K))

    def img_head_major(wT):
        # [D, 512] -> [128, 8192] with [p, m*2048 + d*128 + c] =
        # wT[d*128+p, m*128+c]; head m's chain reads a contiguous 512KB
        return np.ascontiguousarray(
            wT.reshape(ND, 128, HPC, 128).transpose(1, 2, 0, 3).reshape(
                128, ND * 512))

    in_maps = []
    for c in range(NCORE):
        b = c // 4
        o0 = OSL * (c % 4)
        rows = np.concatenate(
            [o0 + h * HD + perm for h in range(HPC)])
        xT = np.ascontiguousarray(x[b].T)  # [D, S]
        # x image: [4*128, ND*512]: [qc*128+p, d*512+t] = xT[d*128+p, qc*512+t]
        xi = xT.reshape(ND, 128, 4, 512).transpose(2, 1, 0, 3).reshape(
            4 * 128, ND * 512)
        # wo image: [128, HPC*D]: [p, e*D+c] = woT[e*128+p, c]
        woT = wo[:, o0:o0 + OSL].T  # [512, D]
        woi = woT.reshape(HPC, 128, D).transpose(1, 0, 2).reshape(128, HPC * D)
        in_maps.append(dict(
            xr=np.ascontiguousarray(xi).astype(BF16),
            wqr=img_head_major(wq[rows].T).astype(BF16),
            wkr=img_head_major(wk[rows].T).astype(BF16),
            wvr=img_dxk(wv[o0:o0 + OSL].T).astype(BF16),
            wor=np.ascontiguousarray(woi).astype(BF16),
            cos2=cos2, sin2=sin2, trim=trim,
        ))
    return in_maps


_NC_CACHE = None


def get_program():
    global _NC_CACHE
    if _NC_CACHE is None:
        _NC_CACHE = _build_program()
    return _NC_CACHE


def run_on_cores(in_maps, trace=False):
    nc = get_program()
    return bass_utils.run_bass_kernel_spmd(
        nc, in_maps, core_ids=list(range(NCORE)), trace=trace)


def kernel(x, freqs_cos, freqs_sin, mask, wq, wk, wv, wo, start_pos=0,
           **_ignored):
    in_maps = _host_prep(x, freqs_cos, freqs_sin, mask, wq, wk, wv, wo)
    res = run_on_cores(in_maps, trace=False)
    outs = [res.results[c]["out"] for c in range(NCORE)]
    full = np.empty((B, S, D), np.float32)
    for b in range(B):
        acc = outs[4 * b].astype(np.float32)
        for c in range(4 * b + 1, 4 * b + 4):
            acc = acc + outs[c]
        full[b] = acc
    return full


# revision 40
# speedup vs baseline: 1.0104x; 1.0104x over previous
"""Trainium2 Bass kernel for causal multi-head attention with RoPE.

Problem: B=2, S=2048, D=2048, H=16 heads (HD=128), fp32 reference.

Sharding (8 NeuronCores): 2-way batch x 4-way heads. Core c handles
batch c//4 and heads 4*(c%4) .. 4*(c%4)+4. Each core computes a partial
output projection over its 512-wide head slice; the host sums the 4
partials per batch element (the row-parallel wo all-reduce).

Measured dead ends (kept for future reference): fp8/DoubleRow on any
value path exceeds the 2e-2 gate (V-only fp8 sims at 0.032 rel err -
softmax concentration defeats error averaging); gpsimd tensor ops and
gpsimd partition_all_reduce are ~3.5us per [128,512] op (too slow for
the den or rope muls); PE warmup matmuls are a wash (the startup is
DMA-ramp-bound, cold-clock matmuls there cost nothing); V-first filler
order and finer startup pieces both measured slower. Run-to-run note:
back-to-back benches trigger a P0 downclock (+60us!) - measure after
a ~60s idle.

Round 3 (cross-phase interleave, vs the ~333us round 2):
  The attention phases are ACT-bound: each 512-wide score block costs
  the PE 432ns (QK^T + PV) but the exp costs ACT ~600ns, so the PE
  leaked ~170ns per block (~96 off-diagonal blocks) plus a 2-3us
  bubble at every chunk boundary waiting for the last head's softmax
  tail. Fix: the NEXT chunk's Q/K/V projection chains (pure PE work)
  are emitted as ~2-matmul "doses" BETWEEN attention blocks, making
  every attention phase PE-bound. Chunk 3's attention has no next
  chunk, so chunk 2's deferred output projection fills it instead.
  Emission order: proj(0) | attn(0)*proj(1) | wo(0) | attn(1)*proj(2)
  | wo(1) | attn(2)*proj(3) | attn(3)*wo(2) | wo(3). x images are
  prefetched two chunks ahead (xpool bufs=2 holds current+next).

Round 2 changes (vs the ~335-339us revision):
  - Denominator merge: the off-diagonal DVE pre-sums fold into the
    diagonal p4d accumulator, so ONE ones-matmul per head-chunk makes
    the whole softmax denominator (-12 PE matmuls, ~2.6us).
  - Startup: rope constants ride the Scalar DMA ring (bypassing the
    Sync FIFO), the first wq/x pieces are 256KB so the first matmul
    issues earlier, and chunk 0 emits all Q chains before any K chain
    so wk/wv/wo stream in behind the Q work.
  - Rope swaps issue from the GpSimd software-DGE ring: the Sync queue
    only carries bulk loads + output rows.
  - Output rows written as two half-rows the moment their evacuations
    land; the last chunk's halves round-robin over the three DMA rings
    (Sync/Scalar/GpSimd) so the final ~2MB drains in parallel.

Baseline notes that still apply: all inputs repacked host-side into
SBUF-image layouts (4-16KB contiguous DMA lines); wq/wk head-major;
causal trim of the diagonal super-blocks; softmax denominator via a
ones-matmul (broadcasts the colsum across partitions); no PE warmup
matmuls (counterproductive - the DMA-bound startup re-throttles HAM).
"""

import math

import numpy as np
import ml_dtypes

import concourse.bass as bass
import concourse.mybir as mybir
import concourse.tile as tile
from concourse import bacc, bass_isa, bass_utils

BF16 = ml_dtypes.bfloat16
F32 = mybir.dt.float32
BF = mybir.dt.bfloat16

B, S, D, H = 2, 2048, 2048, 16
HD = 128
NCORE = 8
HPC = 4            # heads per core
OSL = HPC * HD     # 512-wide output slice per core
NT = S // 128      # 16 token tiles
ND = D // 128      # 16 contraction tiles
NCH = 4            # 512-wide token chunks
SCALE = 1.0 / math.sqrt(HD)


def _build_program():
    nc = bacc.Bacc(
        "TRN2",
        target_bir_lowering=False,
        debug=False,
        enable_asserts=False,
        num_devices=NCORE,
    )
    xr = nc.dram_tensor("xr", [4 * 128, ND * 512], BF, kind="ExternalInput").ap()
    wqr = nc.dram_tensor("wqr", [128, ND * 512], BF, kind="ExternalInput").ap()
    wkr = nc.dram_tensor("wkr", [128, ND * 512], BF, kind="ExternalInput").ap()
    wvr = nc.dram_tensor("wvr", [128, ND * 512], BF, kind="ExternalInput").ap()
    wor = nc.dram_tensor("wor", [128, HPC * D], BF, kind="ExternalInput").ap()
    cos2 = nc.dram_tensor("cos2", [128, S], BF, kind="ExternalInput").ap()
    sin2 = nc.dram_tensor("sin2", [128, S], BF, kind="ExternalInput").ap()
    trim = nc.dram_tensor("trim", [128, 128], BF, kind="ExternalInput").ap()
    out = nc.dram_tensor("out", [S, D], BF, kind="ExternalOutput").ap()

    with tile.TileContext(nc) as tc:
        _kernel_body(tc, xr, wqr, wkr, wvr, wor, cos2, sin2, trim, out)
    nc.compile()
    return nc


def _kernel_body(tc, xr, wqr, wkr, wvr, wor, cos2, sin2, trim, out):
    nc = tc.nc
    Exp = mybir.ActivationFunctionType.Exp

    with (
        tc.tile_pool(name="weights", bufs=1) as wpool,
        tc.tile_pool(name="kv", bufs=1) as kvpool,
        tc.tile_pool(name="consts", bufs=1) as cpool,
        tc.tile_pool(name="qchunk", bufs=2) as qpool,
        tc.tile_pool(name="ctxsb", bufs=2) as ctxpool,
        tc.tile_pool(name="xtp", bufs=2) as xpool,
        tc.tile_pool(name="rope", bufs=3) as rpool,
        tc.tile_pool(name="pg", bufs=6) as ppool,
        tc.tile_pool(name="pgs", bufs=2) as pspool,
        tc.tile_pool(name="den", bufs=2) as dpool,
        tc.tile_pool(name="smallsb", bufs=3) as spool,
        tc.tile_pool(name="outsb", bufs=2) as outpool,
        tc.tile_pool(name="peps", bufs=3, space="PSUM") as peps,
        tc.tile_pool(name="scps", bufs=2, space="PSUM") as scpool,
        tc.tile_pool(name="denps", bufs=1, space="PSUM") as denpool,
        tc.tile_pool(name="ctxps", bufs=2, space="PSUM") as ctxps_pool,
    ):
        wq_s = wpool.tile([128, ND * 512], BF, tag="wq")
        wk_s = wpool.tile([128, ND * 512], BF, tag="wk")
        wv_s = wpool.tile([128, ND * 512], BF, tag="wv")
        wo_s = wpool.tile([128, HPC * D], BF, tag="wo")
        cos_s = cpool.tile([128, S], BF, tag="cos")
        sin_s = cpool.tile([128, S], BF, tag="sin")
        tri_s = cpool.tile([128, 128], BF, tag="trim")
        # ones matrix: den matmul broadcasts the k-colsum to all 128
        # partitions, so no cross-partition broadcast hop is needed.
        ones_s = cpool.tile([128, 128], BF, tag="ones")
        nc.gpsimd.memset(ones_s[:], 1.0)
        # NOTE: no PE warmup matmuls - measured a wash: the startup is
        # DMA-ramp-bound, so the cold (1.2GHz) PE costs nothing there.
        kt = [kvpool.tile([128, S], BF, tag=f"kt{h}", name=f"kt{h}")
              for h in range(HPC)]
        v_s = kvpool.tile([128, NT * 512], BF, tag="v")

        xts = {}
        qts = {qc: [] for qc in range(NCH)}
        ctxTs = {qc: [] for qc in range(NCH)}
        atts = {qc: {} for qc in range(NCH)}

        def xt_dma(qc):
            xt = xpool.tile([128, ND * 512], BF, tag="xt", name=f"xt{qc}")
            xts[qc] = xt
            nc.sync.dma_start(xt[:], xr[qc * 128:(qc + 1) * 128, :])

        def proj_doses(qc, v_first=False):
            """Q/K/V projections + rope for chunk qc, yielded in ~2-MM
            doses so they can interleave the previous chunk's attention
            blocks (pure-PE filler for the ACT-bound softmax). As
            filler (v_first), the V chains go first: their doses carry
            no rope work, so the DVE/ACT-saturated attention window
            gets pure-PE filler and the rope-heavy Q/K tails land in
            the post-attention drain instead."""
            xt = xts[qc]
            ch = slice(qc * 512, (qc + 1) * 512)
            if v_first:
                yield from _v_doses(qc, xt)
            for is_q in (True, False):
                for m in range(HPC):
                    w_s = wq_s if is_q else wk_s
                    nm = "q" if is_q else "k"
                    ps = peps.tile([128, 512], F32, tag="peps",
                                   name=f"ps{nm}{qc}_{m}")
                    for d2 in range(ND // 2):
                        for d in (2 * d2, 2 * d2 + 1):
                            nc.tensor.matmul(
                                ps[:],
                                w_s[:, m * 2048 + d * 128:
                                    m * 2048 + (d + 1) * 128],
                                xt[:, d * 512:(d + 1) * 512],
                                start=(d == 0), stop=(d == ND - 1))
                        yield
                    raw = rpool.tile([128, 512], BF, tag="rraw",
                                     name=f"raw{nm}{qc}_{m}")
                    nc.scalar.copy(raw[:], ps[:])
                    # rope rotate-half: swap 64-partition halves via the
                    # (otherwise idle) GpSimd software-DGE ring - swaps
                    # on the Sync ring head-block the bulk-load FIFO
                    # (measured +20us). The swaps all finish mid-kernel,
                    # so the slow gpsimd teardown DRAIN (~6.5us sweep)
                    # overlaps the tail write drain as long as no LATE
                    # DMA rides gpsimd (hence 2-ring tail writes).
                    swp = rpool.tile([128, 512], BF, tag="rswp",
                                     name=f"swp{nm}{qc}_{m}")
                    nc.gpsimd.dma_start(swp[0:64, :], raw[64:128, :])
                    nc.gpsimd.dma_start(swp[64:128, :], raw[0:64, :])
                    t1 = rpool.tile([128, 512], BF, tag="rt1")
                    nc.vector.tensor_mul(t1[:], raw[:], cos_s[:, ch])
                    nc.vector.tensor_mul(swp[:], swp[:], sin_s[:, ch])
                    if is_q:
                        dst = qpool.tile([128, 512], BF, tag=f"qt{m}",
                                         name=f"qt{m}_{qc}")
                        qts[qc].append(dst)
                        nc.vector.tensor_add(dst[:], t1[:], swp[:])
                    else:
                        nc.vector.tensor_add(kt[m][:, ch], t1[:], swp[:])
                    yield
            if not v_first:
                yield from _v_doses(qc, xt)

        def _v_doses(qc, xt):
            for tt in range(4):
                pv = peps.tile([128, 512], F32, tag="peps",
                               name=f"psv{qc}_{tt}")
                for d2 in range(ND // 2):
                    for d in (2 * d2, 2 * d2 + 1):
                        nc.tensor.matmul(
                            pv[:],
                            xt[:, d * 512 + tt * 128:d * 512 + (tt + 1) * 128],
                            wv_s[:, d * 512:(d + 1) * 512],
                            start=(d == 0), stop=(d == ND - 1))
                    yield
                j = qc * 4 + tt
                nc.vector.tensor_copy(v_s[:, j * 512:(j + 1) * 512], pv[:])
                yield

        def emit_off(qc, h, pump):
            # off-diagonal k-tiles (full 512-wide, no mask)
            noff = 4 * qc
            ctx_ps = ctxps_pool.tile([128, 512], F32, tag="ctxps",
                                     name=f"ctxps{h}_{qc}")
            qt = qts[qc]
            pg_hist = {}
            pair = [None, None]
            accq = None
            for j in range(noff):
                sc = scpool.tile([128, 512], F32, tag="sc",
                                 name=f"sc{h}_{qc}_{j}")
                nc.tensor.matmul(
                    sc[:], kt[h][:, j * 128:(j + 1) * 128],
                    qt[h][:], start=True, stop=True)
                pg = ppool.tile([128, 512], BF, tag="pg",
                                name=f"pg{h}_{qc}_{j}")
                nc.scalar.activation(pg[:], sc[:], Exp, scale=SCALE)
                # the filler dose sits BETWEEN the scores matmul and the
                # PV matmul in the PE queue, so the exp latency is hidden
                # behind it instead of stalling the in-order PE
                pump()
                nc.tensor.matmul(
                    ctx_ps[:],
                    v_s[:, j * 512 + h * 128:j * 512 + (h + 1) * 128],
                    pg[:], start=(j == 0), stop=False)
                # denominator pre-sums on DVE: pairs -> quads -> fold
                pg_hist[j] = pg
                if j % 2 == 1:
                    pp = pspool.tile([128, 512], BF,
                                     tag=f"pgs{(j // 2) % 2}",
                                     name=f"pgs{h}_{qc}_{j}")
                    nc.vector.tensor_add(pp[:], pg_hist[j - 1][:], pg[:])
                    pair[(j // 2) % 2] = pp
                if j % 4 == 3:
                    p4 = pspool.tile([128, 512], BF, tag="pgs4",
                                     name=f"pgs4_{h}_{qc}_{j}")
                    nc.vector.tensor_add(p4[:], pair[0][:], pair[1][:])
                    if accq is None:
                        accq = p4
                    else:
                        nacc = pspool.tile([128, 512], BF, tag="pgs8",
                                           name=f"pgs8_{h}_{qc}_{j}")
                        nc.vector.tensor_add(nacc[:], accq[:], p4[:])
                        accq = nacc
            atts[qc][h] = (ctx_ps, accq)

        def emit_diag(qc, h, pump):
            # diagonal k-tiles, causally trimmed: k-tile r only sees
            # q columns >= 128r; the first 128 of those are the
            # triangular boundary block (element mask).
            noff = 4 * qc
            ctx_ps, accq = atts[qc][h]
            qt = qts[qc]
            den_ps = denpool.tile([128, 512], F32, tag="den",
                                  name=f"den{h}_{qc}")
            p4d = dpool.tile([128, 512], BF, tag="p4d",
                             name=f"p4d{h}_{qc}")
            for r in range(4):
                off = 128 * r
                j = noff + r
                sc = scpool.tile([128, 512], F32, tag="sc",
                                 name=f"scd{h}_{qc}_{r}")
                nc.tensor.matmul(
                    sc[:, off:512], kt[h][:, j * 128:(j + 1) * 128],
                    qt[h][:, off:512], start=True, stop=True)
                pg = ppool.tile([128, 512], BF, tag="pg",
                                name=f"pgd{h}_{qc}_{r}")
                nc.scalar.activation(pg[:, off:512], sc[:, off:512],
                                     Exp, scale=SCALE)
                nc.vector.tensor_mul(pg[:, off:off + 128],
                                     pg[:, off:off + 128], tri_s[:])
                pump()
                nc.tensor.matmul(
                    ctx_ps[:, off:512],
                    v_s[:, j * 512 + h * 128:j * 512 + (h + 1) * 128],
                    pg[:, off:512],
                    start=(qc == 0 and r == 0), stop=(r == 3))
                if r == 0:
                    # fold the off-diagonal DVE pre-sums in here so a
                    # single ones-matmul produces the whole denominator
                    if accq is None:
                        nc.vector.tensor_copy(p4d[:], pg[:])
                    else:
                        nc.vector.tensor_add(p4d[:], accq[:], pg[:])
                else:
                    nc.vector.tensor_add(p4d[:, off:512],
                                         p4d[:, off:512], pg[:, off:512])
            nc.tensor.matmul(den_ps[:], ones_s[:], p4d[:],
                             start=True, stop=True)
            # softmax normalization folded into ctx eviction; den is
            # already broadcast across partitions by the ones matmul
            rbc = spool.tile([128, 512], F32, tag="rbc")
            nc.vector.reciprocal_approx_fast(rbc[:], den_ps[:])
            ctx = ctxpool.tile([128, 512], BF, tag=f"ctx{h}",
                               name=f"ctxT{h}_{qc}")
            ctxTs[qc].append(ctx)
            nc.vector.tensor_mul(ctx[:], ctx_ps[:], rbc[:])
            pump()
            pump()

        # only hardware-DGE rings: a single gpsimd (software-DGE) DMA
        # costs ~6.5us of teardown DRAIN at kernel end
        rings = [nc.sync, nc.scalar]

        def emit_wo(qc):
            # staged output projection: each group's first 3 head-MMs
            # are emitted STAG groups ahead of its final head-3 MM. On
            # the last chunk the row halves round-robin over the three
            # DMA rings so the final ~2MB drains in parallel.
            ctxT = ctxTs[qc]
            STAG = 4 if qc < NCH - 1 else 5
            groups = [(tl, dc) for tl in range(4) for dc in range(4)]
            opst = {}
            osbt = {}
            nring = [0]

            def write_half(tt, half, osb):
                cs = slice(half * 1024, (half + 1) * 1024)
                if qc == NCH - 1 and tt == 4 * qc + 3:
                    # final row: partition-split each half across both
                    # rings so the very last piece drains in ~half the
                    # time (drain cost is per-packet, 128 lines/half)
                    nc.sync.dma_start(
                        out[tt * 128:tt * 128 + 64, cs], osb[0:64, cs])
                    nc.scalar.dma_start(
                        out[tt * 128 + 64:(tt + 1) * 128, cs],
                        osb[64:128, cs])
                    return
                eng = rings[nring[0] % 2] if qc == NCH - 1 else nc.sync
                nring[0] += 1
                eng.dma_start(
                    out[tt * 128:(tt + 1) * 128, cs], osb[:, cs])

            for gi in range(16 + STAG):
                if gi >= STAG:
                    tl, dc = groups[gi - STAG]
                    tt = 4 * qc + tl
                    ops = opst[gi - STAG]
                    osb = osbt[tl]
                    nc.tensor.matmul(
                        ops[:], ctxT[3][:, tl * 128:(tl + 1) * 128],
                        wo_s[:, 3 * D + dc * 512:3 * D + (dc + 1) * 512],
                        start=False, stop=True)
                    if dc % 2 == 0:
                        nc.vector.tensor_copy(
                            osb[:, dc * 512:(dc + 1) * 512], ops[:])
                    else:
                        nc.scalar.copy(
                            osb[:, dc * 512:(dc + 1) * 512], ops[:])
                    if dc == 1:
                        write_half(tt, 0, osb)
                    elif dc == 3:
                        write_half(tt, 1, osb)
                if gi < 16:
                    tl, dc = groups[gi]
                    if dc == 0:
                        osbt[tl] = outpool.tile([128, D], BF, tag="osb",
                                                name=f"osb{4 * qc + tl}")
                    # pool choice by release time: peps is free at wo
                    # start; sc and ctx tiles release progressively
                    # through the diag-3 tail.
                    if qc == NCH - 1:
                        sel = (peps, "peps") if gi % 3 == 0 else (
                            (scpool, "sc") if gi % 3 == 1 else
                            (ctxps_pool, "ctxps"))
                    elif gi == 0:
                        sel = (peps, "peps")
                    elif gi % 2 == 0:
                        sel = (scpool, "sc")
                    else:
                        sel = (ctxps_pool, "ctxps")
                    ops = sel[0].tile([128, 512], F32, tag=sel[1],
                                      name=f"ops{qc}_{tl}_{dc}")
                    opst[gi] = ops
                    for e in range(3):
                        nc.tensor.matmul(
                            ops[:], ctxT[e][:, tl * 128:(tl + 1) * 128],
                            wo_s[:, e * D + dc * 512:e * D + (dc + 1) * 512],
                            start=(e == 0), stop=False)

        def wo_doses(qc):
            """Output projection of chunk qc in ~2-MM doses (filler for
            the LAST chunk's attention, whose next-proj doesn't exist).
            All evacuations ride the DVE - ACT is saturated with exps.
            ctx for all 4 heads is long since ready, so each group runs
            its 4 MMs straight through on the (now idle) proj pool."""
            ctxT = ctxTs[qc]
            for tl in range(4):
                osb = outpool.tile([128, D], BF, tag="osb",
                                   name=f"osb{4 * qc + tl}")
                tt = 4 * qc + tl
                for dc in range(4):
                    ops = peps.tile([128, 512], F32, tag="peps",
                                    name=f"ops{qc}_{tl}_{dc}")
                    for e in range(4):
                        nc.tensor.matmul(
                            ops[:], ctxT[e][:, tl * 128:(tl + 1) * 128],
                            wo_s[:, e * D + dc * 512:e * D + (dc + 1) * 512],
                            start=(e == 0), stop=(e == 3))
                        yield
                    nc.vector.tensor_copy(
                        osb[:, dc * 512:(dc + 1) * 512], ops[:])
                    if dc == 1:
                        nc.sync.dma_start(
                            out[tt * 128:(tt + 1) * 128, 0:1024],
                            osb[:, 0:1024])
                    elif dc == 3:
                        nc.sync.dma_start(
                            out[tt * 128:(tt + 1) * 128, 1024:2048],
                            osb[:, 1024:2048])
                    yield

        # ---- startup loads -----------------------------------------
        # Small leading pieces get the first matmul started early; the
        # rope constants ride the Scalar DMA ring, bypassing the Sync
        # FIFO of bulk loads. Q chains are emitted before K chains so
        # wk/wv/wo/x1 can stream in behind the Q work.
        xt0 = xpool.tile([128, ND * 512], BF, tag="xt", name="xt0")
        xts[0] = xt0
        nc.scalar.dma_start(cos_s[:], cos2[:])
        nc.scalar.dma_start(sin_s[:], sin2[:])
        nc.scalar.dma_start(tri_s[:], trim[:])
        nc.sync.dma_start(wq_s[:, 0:1024], wqr[:, 0:1024])
        nc.sync.dma_start(xt0[:, 0:1024], xr[0:128, 0:1024])
        nc.sync.dma_start(wq_s[:, 1024:2048], wqr[:, 1024:2048])
        nc.sync.dma_start(xt0[:, 1024:2048], xr[0:128, 1024:2048])
        nc.sync.dma_start(xt0[:, 2048:4096], xr[0:128, 2048:4096])
        nc.sync.dma_start(wq_s[:, 2048:4096], wqr[:, 2048:4096])
        nc.sync.dma_start(xt0[:, 4096:6144], xr[0:128, 4096:6144])
        nc.sync.dma_start(xt0[:, 6144:8192], xr[0:128, 6144:8192])
        nc.sync.dma_start(wq_s[:, 4096:6144], wqr[:, 4096:6144])
        nc.sync.dma_start(wq_s[:, 6144:8192], wqr[:, 6144:8192])
        nc.sync.dma_start(wk_s[:, 0:2048], wkr[:, 0:2048])
        nc.sync.dma_start(wk_s[:, 2048:4096], wkr[:, 2048:4096])
        nc.sync.dma_start(wv_s[:, 0:4096], wvr[:, 0:4096])
        nc.sync.dma_start(wk_s[:, 4096:6144], wkr[:, 4096:6144])
        nc.sync.dma_start(wk_s[:, 6144:8192], wkr[:, 6144:8192])
        nc.sync.dma_start(wv_s[:, 4096:8192], wvr[:, 4096:8192])
        nc.sync.dma_start(wo_s[:], wor[:])
        xt_dma(1)

        # ---- chunk 0 projections: bulk, DMA-paced ------------------
        for _ in proj_doses(0):
            pass

        # ---- pipelined chunks --------------------------------------
        for qc in range(NCH):
            if qc < NCH - 1:
                if qc + 2 < NCH:
                    xt_dma(qc + 2)
                filler = proj_doses(qc + 1)
            else:
                filler = wo_doses(2)

            def pump(f=filler):
                next(f, None)

            for h in range(HPC):
                emit_off(qc, h, pump)
                emit_diag(qc, h, pump)
            for _ in filler:
                pass
            if qc != 2:
                emit_wo(qc)


def _host_prep(x, freqs_cos, freqs_sin, mask, wq, wk, wv, wo):
    """Build per-core input dicts (SBUF-image layouts, bf16)."""
    x = np.asarray(x, np.float32)
    wq = np.asarray(wq, np.float32)
    wk = np.asarray(wk, np.float32)
    wv = np.asarray(wv, np.float32)
    wo = np.asarray(wo, np.float32)
    cos = np.asarray(freqs_cos, np.float32)
    sin = np.asarray(freqs_sin, np.float32)

    perm = np.concatenate([np.arange(0, HD, 2), np.arange(1, HD, 2)])
    cos2 = np.ascontiguousarray(
        np.concatenate([cos.T, cos.T], axis=0)).astype(BF16)
    sin2 = np.ascontiguousarray(
        np.concatenate([-sin.T, sin.T], axis=0)).astype(BF16)

    # triangular boundary mask: T[kl, ql] = 1 iff ql >= kl
    kl = np.arange(128)[:, None]
    ql = np.arange(128)[None, :]
    trim = (ql >= kl).astype(np.float32).astype(BF16)

    def img_dxk(wT):
        # [D, K] -> [128, ND*K] with [p, d*K+c] = wT[d*128+p, c]
        Dd, K = wT.shape
        return np.ascontiguousarray(
            wT.reshape(ND, 128, K).transpose(1, 0, 2).reshape(128, ND * K))

    def img_head_major(wT):
        # [D, 512] -> [128, 8192] with [p, m*2048 + d*128 + c] =
        # wT[d*128+p, m*128+c]; head m's chain reads a contiguous 512KB
        return np.ascontiguousarray(
            wT.reshape(ND, 128, HPC, 128).transpose(1, 2, 0, 3).reshape(
                128, ND * 512))

    in_maps = []
    for c in range(NCORE):
        b = c // 4
        o0 = OSL * (c % 4)
        rows = np.concatenate(
            [o0 + h * HD + perm for h in range(HPC)])
        xT = np.ascontiguousarray(x[b].T)  # [D, S]
        # x image: [4*128, ND*512]: [qc*128+p, d*512+t] = xT[d*128+p, qc*512+t]
        xi = xT.reshape(ND, 128, 4, 512).transpose(2, 1, 0, 3).reshape(
            4 * 128, ND * 512)
        # wo image: [128, HPC*D]: [p, e*D+c] = woT[e*128+p, c]
        woT = wo[:, o0:o0 + OSL].T  # [512, D]
        woi = woT.reshape(HPC, 128, D).transpose(1, 0, 2).reshape(128, HPC * D)
        in_maps.append(dict(
            xr=np.ascontiguousarray(xi).astype(BF16),
            wqr=img_head_major(wq[rows].T).astype(BF16),
            wkr=img_head_major(wk[rows].T).astype(BF16),
            wvr=img_dxk(wv[o0:o0 + OSL].T).astype(BF16),
            wor=np.ascontiguousarray(woi).astype(BF16),
            cos2=cos2, sin2=sin2, trim=trim,
        ))
    return in_maps


_NC_CACHE = None


def get_program():
    global _NC_CACHE
    if _NC_CACHE is None:
        _NC_CACHE = _build_program()
    return _NC_CACHE


def run_on_cores(in_maps, trace=False):
    nc = get_program()
    return bass_utils.run_bass_kernel_spmd(
        nc, in_maps, core_ids=list(range(NCORE)), trace=trace)


def kernel(x, freqs_cos, freqs_sin, mask, wq, wk, wv, wo, start_pos=0,
           **_ignored):
    in_maps = _host_prep(x, freqs_cos, freqs_sin, mask, wq, wk, wv, wo)
    res = run_on_cores(in_maps, trace=False)
    outs = [res.results[c]["out"] for c in range(NCORE)]
    full = np.empty((B, S, D), np.float32)
    for b in range(B):
        acc = outs[4 * b].astype(np.float32)
        for c in range(4 * b + 1, 4 * b + 4):
            acc = acc + outs[c]
        full[b] = acc
    return full
